# revision 3
# baseline (speedup 1.0000x reference)
"""DenseGraphAttentionHead Trainium2 Bass kernel (8-core SPMD row-sharded).

reference math:
    Wh = nodes @ W_w.T + W_b                    [N, 256]
    Wh1 = Wh @ a1_w.T + a1_b                    [N, 1]
    Wh2 = Wh @ a2_w.T + a2_b                    [N, 1]
    scores = leaky_relu(Wh1 + Wh2.T, 0.2)       [N, N]
    attention = softmax(where(edge, scores, -inf), axis=1)
    out = attention @ Wh                        [N, 256]

Key identities (softmax over j is invariant to per-row(i) factors):
    p[i] = exp(0.8*Wh1[i]),  r[j] = exp(Wh2[j]),  e[j] = exp(-0.8*Wh2[j])
    exp(lrelu(Wh1+Wh2) - 0.2*Wh1) = r[j] * max(e[j], p[i])
so, folding r[j] into the value rows (wh_aug[j,:] = r[j]*Wh[j,:], and the
softmax-denominator column holds r[j] instead of 1):
    attention_ij @ Wh = (sum_j edge_ij*max(e_j,p_i) * r_j*Wh_j)
                      / (sum_j edge_ij*max(e_j,p_i) * r_j)
The dense [N,N] elementwise work collapses to ONE fused DVE
scalar_tensor_tensor per 128x512 tile: X = (p_b MAX e_j) MULT mask01,
with the mask kept as {0,1} fp8 in HBM (upcast to fp16 by the SWDGE DMA).

Per core c (rows i in [c*1024, (c+1)*1024)):
  nodesT is column-rolled per core so chunk order starts at the core's own
  block; mask rows are rolled identically, so SPMD code is rank-oblivious.
  psum[i, 0:257] += X[:, i_blk].T @ wh_aug over j chunks; col 256 = softmax
  denominator. out = psum[:, :256]/denom + W_b (softmax rows sum to 1, so
  the +W_b bias commutes with attention@).
"""
import sys
import types

import numpy as np

N_NODES = 8192
IN_DIM = 512
OUT_DIM = 256
ALPHA = 0.2
N_CORES = 8
ROWS = N_NODES // N_CORES          # 1024 rows per core
NCK = N_NODES // 128               # 64 j-chunks of 128
NBLK = 8                           # j-blocks of 8 chunks (1024 nodes)
GRP = 4                            # j-chunks per mask-DMA batch
HALF = 512
NG = NCK // GRP                    # 16 groups of GRP chunks per i-half
WAUG = OUT_DIM + 1                 # 256 value cols + denominator col
WARM_MMS = 56                      # PE warm-up dummy matmuls

_CACHE = {}


def _ensure_ntff_hook():
    """antenv.axon_hooks is absent in this container; shim it so
    run_bass_kernel_spmd(trace=True) can reach the NTFF profiler."""
    if "antenv.axon_hooks" in sys.modules:
        return
    holder = [None]
    mod = types.ModuleType("antenv.axon_hooks")
    mod.set_axon_ntff_profile_hook = lambda h: holder.__setitem__(0, h)
    mod.get_axon_ntff_profile_hook = lambda: holder[0]
    sys.modules["antenv.axon_hooks"] = mod
    try:
        from trn_agent_boot.trn_boot import _ntff_profile_via_ctypes
        mod.set_axon_ntff_profile_hook(
            _ntff_profile_via_ctypes("/opt/axon/libaxon_pjrt.so"))
    except Exception:
        pass


def _build_nc():
    import concourse.bacc as bacc
    import concourse.tile as tile
    from concourse import mybir

    F16 = mybir.dt.float16
    F32 = mybir.dt.float32
    MULT = mybir.AluOpType.mult
    MAX = mybir.AluOpType.max
    ADD = mybir.AluOpType.add
    EXP = mybir.ActivationFunctionType.Exp
    COPY = mybir.ActivationFunctionType.Copy

    nc = bacc.Bacc("TRN2", target_bir_lowering=False, debug=False,
                   num_devices=N_CORES)

    nodesT_d = nc.dram_tensor("nodesT", [IN_DIM, N_NODES], F16,
                              kind="ExternalInput")
    maskm_d = nc.dram_tensor("maskm", [N_NODES, ROWS], mybir.dt.float8e4,
                             kind="ExternalInput")
    wtaug_d = nc.dram_tensor("wt_aug", [IN_DIM, WAUG], F16,
                             kind="ExternalInput")
    v1_d = nc.dram_tensor("v1", [IN_DIM, 1], F16, kind="ExternalInput")
    wb_d = nc.dram_tensor("wb_bc", [128, OUT_DIM], F32, kind="ExternalInput")
    cb_d = nc.dram_tensor("cb", [128, 3], F32, kind="ExternalInput")
    out_d = nc.dram_tensor("out", [ROWS, OUT_DIM], F32, kind="ExternalOutput")

    with tile.TileContext(nc) as tc:
        with (
            tc.tile_pool(name="consts", bufs=1) as consts,
            tc.tile_pool(name="ndpool", bufs=3) as ndpool,
            tc.tile_pool(name="grpp", bufs=4) as grpp,
            tc.tile_pool(name="outp", bufs=2) as outp,
        ):
            # ---- constant / persistent tiles ----
            wt_t = []
            v1_t = []
            for d4 in range(4):
                w = consts.tile([128, WAUG], F16, name=f"wt{d4}",
                                tag=f"wt{d4}")
                nc.scalar.dma_start(w[:], wtaug_d[d4 * 128:(d4 + 1) * 128, :])
                wt_t.append(w)
                v = consts.tile([128, 1], F16, name=f"v1_{d4}", tag=f"v1_{d4}")
                nc.sync.dma_start(v[:], v1_d[d4 * 128:(d4 + 1) * 128, :])
                v1_t.append(v[:])
            wb_bc = consts.tile([128, OUT_DIM], F32)
            nc.scalar.dma_start(wb_bc[:], wb_d[:])
            cb = consts.tile([128, 3], F32)
            nc.sync.dma_start(cb[:], cb_d[:])

            warm_a = consts.tile([128, 64], F16)
            nc.gpsimd.memset(warm_a[:], 0.0)

            wh_aug = consts.tile([128, NCK, WAUG], F16)
            e128 = consts.tile([128, NCK], F32)
            r128 = consts.tile([128, NCK], F32)
            p_row = consts.tile([1, ROWS], F16)
            p_b = consts.tile([128, ROWS], F16)

            with (
                tc.tile_pool(name="psW", bufs=1, space="PSUM") as psW,
                tc.tile_pool(name="psH", bufs=3, space="PSUM") as psH,
                tc.tile_pool(name="psB", bufs=1, space="PSUM") as psB,
            ):
                # ---- PE warm-up: dummy matmuls on zeroed scratch ----
                warm_ps = psW.tile([1, HALF], F32, name="warm_ps", tag="pw1")
                for k in range(WARM_MMS):
                    nc.tensor.matmul(
                        warm_ps[:, 0:64], warm_a[:, 0:1], warm_a[:, 0:64],
                        start=(k == 0), stop=(k == WARM_MMS - 1),
                        skip_group_check=True)

                # ---- block-0 nodes (own rows, rolled to front) ----
                ndT_tiles = {}

                def dma_ndT(b):
                    ndT = ndpool.tile([128, 4, ROWS], F16, name="ndT",
                                      tag="ndT")
                    for d4 in range(4):
                        eng = nc.sync if d4 < 2 else nc.scalar
                        eng.dma_start(
                            ndT[:, d4, :],
                            nodesT_d[d4 * 128:(d4 + 1) * 128,
                                     b * ROWS:(b + 1) * ROWS])
                    ndT_tiles[b] = ndT

                dma_ndT(0)
                dma_ndT(1)
                ndT0 = ndT_tiles[0]

                # ---- Wh1 row for own block; p = exp(0.8*Wh1) ----
                for h2 in range(2):
                    pw1 = psW.tile([1, HALF], F32, name="pw1", tag="pw1")
                    for d4 in range(4):
                        nc.tensor.matmul(
                            pw1[:], v1_t[d4],
                            ndT0[:, d4, h2 * HALF:(h2 + 1) * HALF],
                            start=(d4 == 0), stop=(d4 == 3),
                            skip_group_check=True)
                    nc.scalar.activation(
                        p_row[:, h2 * HALF:(h2 + 1) * HALF], pw1[:], EXP,
                        scale=1.0 - ALPHA, bias=cb[0:1, 0:1])
                    nc.gpsimd.partition_broadcast(
                        p_b[:, h2 * HALF:(h2 + 1) * HALF],
                        p_row[:, h2 * HALF:(h2 + 1) * HALF])

                def build_block(b, dve_copy=False):
                    ndT = ndT_tiles.pop(b)
                    for ckl in range(8):
                        ck = b * 8 + ckl
                        pwh = psH.tile([128, WAUG], F32, name="pwh",
                                       tag="pwh", bufs=3)
                        for d4 in range(4):
                            nc.tensor.matmul(
                                pwh[:],
                                ndT[:, d4, ckl * 128:(ckl + 1) * 128],
                                wt_t[d4][:],
                                start=(d4 == 0), stop=(d4 == 3),
                                skip_group_check=True)
                        # col 256 of pwh = Wh2 (wt_aug col 256 = W.T@a2)
                        nc.scalar.activation(
                            e128[:, ck:ck + 1], pwh[:, 256:257], EXP,
                            scale=-(1.0 - ALPHA), bias=cb[:, 1:2])
                        nc.scalar.activation(
                            r128[:, ck:ck + 1], pwh[:, 256:257], EXP,
                            scale=1.0, bias=cb[:, 2:3])
                        if dve_copy:
                            nc.vector.tensor_scalar(
                                wh_aug[:, ck, 0:OUT_DIM], pwh[:, 0:OUT_DIM],
                                r128[:, ck:ck + 1], None, op0=MULT)
                        else:
                            nc.scalar.activation(
                                wh_aug[:, ck, 0:OUT_DIM], pwh[:, 0:OUT_DIM],
                                COPY, scale=r128[:, ck:ck + 1])
                    # denominator column = r_j (batched, strided fp16 write)
                    nc.vector.tensor_copy(
                        wh_aug[:, b * 8:b * 8 + 8, OUT_DIM:WAUG],
                        r128[:, b * 8:b * 8 + 8])

                build_block(0, dve_copy=True)
                dma_ndT(2)
                build_block(1, dve_copy=True)

                def emit_x(h, g, suffix="", bufs=4):
                    mgrp = grpp.tile([128, GRP, HALF], F16,
                                     name=f"mgrp{suffix}",
                                     tag=f"mgrp{suffix}", bufs=bufs)
                    msrc = maskm_d[g * GRP * 128:(g + 1) * GRP * 128,
                                   h * HALF:(h + 1) * HALF]
                    msrc = msrc.rearrange("(c p) i -> p c i", p=128)
                    nc.gpsimd.dma_start(mgrp[:], msrc)  # fp8->fp16 cast
                    xgrp = grpp.tile([128, GRP, HALF], F16,
                                     name=f"xgrp{suffix}",
                                     tag=f"xgrp{suffix}", bufs=bufs)
                    for ckl in range(GRP):
                        ck = g * GRP + ckl
                        nc.vector.scalar_tensor_tensor(
                            xgrp[:, ckl, :],
                            p_b[:, h * HALF:(h + 1) * HALF],
                            e128[:, ck:ck + 1],
                            mgrp[:, ckl, :],
                            op0=MAX, op1=MULT)
                    return xgrp

                def readout(h, accs):
                    for ib in range(4):
                        recip = outp.tile([128, 1], F32, name="recip",
                                          tag="recip")
                        nc.vector.reciprocal(recip[:],
                                             accs[ib][:, OUT_DIM:WAUG])
                        o = outp.tile([128, OUT_DIM], F32, name="o", tag="o")
                        nc.vector.scalar_tensor_tensor(
                            o[:], accs[ib][:, 0:OUT_DIM], recip[:], wb_bc[:],
                            op0=MULT, op1=ADD)
                        r0 = h * HALF + ib * 128
                        eng = nc.sync if ib % 2 == 0 else nc.scalar
                        eng.dma_start(out_d[r0:r0 + 128, :], o[:])

                # build schedule: block b DMA'd 2 groups early, built at
                # group 2b-3 (b>=2); blocks 0/1 pre-built above.
                prefetched = []
                for h in range(2):
                    accs = [psB.tile([128, WAUG], F32, name=f"acc{ib}",
                                     tag=f"acc{ib}") for ib in range(4)]
                    for g in range(NG):
                        if h == 0 and g < 6:
                            if g + 3 <= 7:
                                dma_ndT(g + 3)
                            build_block(g + 2)
                        if h == 1 and g < len(prefetched):
                            xgrp = prefetched[g]
                        else:
                            xgrp = emit_x(h, g)
                        last_g = (g == NG - 1)
                        if last_g:
                            # ib-outer so acc banks retire in order
                            for ib in range(4):
                                for ckl in range(GRP):
                                    ck = g * GRP + ckl
                                    nc.tensor.matmul(
                                        accs[ib][:],
                                        xgrp[:, ckl, ib * 128:(ib + 1) * 128],
                                        wh_aug[:, ck, :],
                                        start=False, stop=(ckl == GRP - 1),
                                        skip_group_check=True)
                        else:
                            for ckl in range(GRP):
                                ck = g * GRP + ckl
                                for ib in range(4):
                                    nc.tensor.matmul(
                                        accs[ib][:],
                                        xgrp[:, ckl, ib * 128:(ib + 1) * 128],
                                        wh_aug[:, ck, :],
                                        start=(ck == 0), stop=False,
                                        skip_group_check=True)
                    if h == 0:
                        # build h=1's first two score groups while the DVE
                        # is idle at the tail of h=0
                        prefetched = [emit_x(1, 0, "p", 2),
                                      emit_x(1, 1, "p", 2)]
                    readout(h, accs)
    nc.compile()
    return nc


def _get_nc():
    if "nc" not in _CACHE:
        _CACHE["nc"] = _build_nc()
    return _CACHE["nc"]


def _prep_in_maps(nodes, edge_mat, W_w, W_b, a1_w, a1_b, a2_w, a2_b):
    f16 = np.float16
    nodes = np.asarray(nodes, dtype=np.float32)
    edge_mat = np.asarray(edge_mat, dtype=bool)
    W_w = np.asarray(W_w, dtype=np.float32)
    W_b = np.asarray(W_b, dtype=np.float32)
    a1_w = np.asarray(a1_w, dtype=np.float32)
    a1_b = np.asarray(a1_b, dtype=np.float32)
    a2_w = np.asarray(a2_w, dtype=np.float32)
    a2_b = np.asarray(a2_b, dtype=np.float32)

    nodesT = np.ascontiguousarray(nodes.T).astype(f16)          # [512, 8192]
    v1 = (W_w.T @ a1_w[0]).astype(f16)[:, None]                 # [512, 1]
    v2 = (W_w.T @ a2_w[0]).astype(f16)[:, None]
    wt_aug = np.concatenate([W_w.T.astype(f16), v2], axis=1)    # [512, 257]
    c1v = float(W_b @ a1_w[0]) + float(a1_b[0])
    c2v = float(W_b @ a2_w[0]) + float(a2_b[0])
    # cb col0: bias for p = exp(0.8*(pw1 + c1)); col1: e bias; col2: r bias
    cb = np.broadcast_to(
        np.array([(1.0 - ALPHA) * c1v, -(1.0 - ALPHA) * c2v, c2v],
                 np.float32)[None, :], (128, 3)).copy()
    wb_bc = np.ascontiguousarray(
        np.broadcast_to(W_b[None, :], (128, OUT_DIM))).astype(np.float32)
    # multiplicative {0,1} mask, transposed, fp8 (cast to fp16 during DMA)
    import ml_dtypes
    maskT = np.where(edge_mat, 1, 0).astype(ml_dtypes.float8_e4m3fn).T

    in_maps = []
    for c in range(N_CORES):
        rs = c * ROWS
        sl = slice(rs, rs + ROWS)
        # roll node columns / mask rows so each core's own block is first
        nodesT_c = np.ascontiguousarray(
            np.concatenate([nodesT[:, rs:], nodesT[:, :rs]], axis=1))
        maskm_c = np.ascontiguousarray(
            np.concatenate([maskT[rs:, sl], maskT[:rs, sl]], axis=0))
        in_maps.append({
            "nodesT": nodesT_c,
            "maskm": maskm_c,
            "wt_aug": wt_aug,
            "v1": v1,
            "wb_bc": wb_bc,
            "cb": cb,
        })
    return in_maps


def _run(inputs, trace=False, trace_cores=None):
    from concourse.bass_utils import run_bass_kernel_spmd
    if trace:
        _ensure_ntff_hook()
    nc = _get_nc()
    in_maps = _prep_in_maps(**inputs)
    res = run_bass_kernel_spmd(nc, in_maps, list(range(N_CORES)),
                               trace=trace, trace_cores=trace_cores)
    out = np.concatenate([res.results[c]["out"] for c in range(N_CORES)],
                         axis=0)
    return out, res


def kernel(**inputs) -> np.ndarray:
    out, _ = _run(inputs, trace=False)
    return out


# revision 4
# speedup vs baseline: 1.0168x; 1.0168x over previous
"""DenseGraphAttentionHead Trainium2 Bass kernel (8-core SPMD row-sharded).

reference math:
    Wh = nodes @ W_w.T + W_b                    [N, 256]
    Wh1 = Wh @ a1_w.T + a1_b                    [N, 1]
    Wh2 = Wh @ a2_w.T + a2_b                    [N, 1]
    scores = leaky_relu(Wh1 + Wh2.T, 0.2)       [N, N]
    attention = softmax(where(edge, scores, -inf), axis=1)
    out = attention @ Wh                        [N, 256]

Key identities (softmax over j is invariant to per-row(i) factors):
    p[i] = exp(0.8*Wh1[i]),  r[j] = exp(Wh2[j]),  e[j] = exp(-0.8*Wh2[j])
    exp(lrelu(Wh1+Wh2) - 0.2*Wh1) = r[j] * max(e[j], p[i])
so, folding r[j] into the value rows (wh_aug[j,:] = r[j]*Wh[j,:], and the
softmax-denominator column holding r[j] instead of 1):
    attention_ij @ Wh = (sum_j edge_ij*max(e_j,p_i) * r_j*Wh_j)
                      / (sum_j edge_ij*max(e_j,p_i) * r_j)
The dense [N,N] elementwise work collapses to ONE fused DVE
scalar_tensor_tensor per 128x512 tile: X = (p_b MAX e_j) MULT mask01,
with the mask kept as {0,1} fp8 in HBM (upcast to fp16 by the SWDGE DMA).

Per core c (rows i in [c*1024, (c+1)*1024)):
  nodesT is column-rolled per core so chunk order starts at the core's own
  block; mask rows are rolled identically, so SPMD code is rank-oblivious.
  psum[i, 0:258] += X[:, i_blk].T @ wh_aug over j chunks; col 256 = softmax
  denominator (col 257 is a dead rider). out = psum[:, :256]/denom + W_b
  (softmax rows sum to 1, so the +W_b bias commutes with attention@).
"""
import sys
import types

import numpy as np

N_NODES = 8192
IN_DIM = 512
OUT_DIM = 256
ALPHA = 0.2
N_CORES = 8
ROWS = N_NODES // N_CORES          # 1024 rows per core
NCK = N_NODES // 128               # 64 j-chunks of 128
NBLK = 8                           # j-blocks of 8 chunks (1024 nodes)
GRP = 4                            # j-chunks per mask-DMA batch
HALF = 512
NG = NCK // GRP                    # 16 groups of GRP chunks per i-half
WAUG = OUT_DIM + 2                 # 256 value cols + denom col + Wh2 rider
WTW = OUT_DIM + 4                  # wt cols: 256 W + 0 + v2 + v1 + pad
WARM_MMS = 56                      # PE warm-up dummy matmuls

_CACHE = {}


def _ensure_ntff_hook():
    """antenv.axon_hooks is absent in this container; shim it so
    run_bass_kernel_spmd(trace=True) can reach the NTFF profiler."""
    if "antenv.axon_hooks" in sys.modules:
        return
    holder = [None]
    mod = types.ModuleType("antenv.axon_hooks")
    mod.set_axon_ntff_profile_hook = lambda h: holder.__setitem__(0, h)
    mod.get_axon_ntff_profile_hook = lambda: holder[0]
    sys.modules["antenv.axon_hooks"] = mod
    try:
        from trn_agent_boot.trn_boot import _ntff_profile_via_ctypes
        mod.set_axon_ntff_profile_hook(
            _ntff_profile_via_ctypes("/opt/axon/libaxon_pjrt.so"))
    except Exception:
        pass


def _build_nc():
    import concourse.bacc as bacc
    import concourse.tile as tile
    from concourse import mybir

    F16 = mybir.dt.float16
    F32 = mybir.dt.float32
    MULT = mybir.AluOpType.mult
    MAX = mybir.AluOpType.max
    ADD = mybir.AluOpType.add
    EXP = mybir.ActivationFunctionType.Exp
    COPY = mybir.ActivationFunctionType.Copy

    nc = bacc.Bacc("TRN2", target_bir_lowering=False, debug=False,
                   num_devices=N_CORES)

    nodesT_d = nc.dram_tensor("nodesT", [IN_DIM, N_NODES], F16,
                              kind="ExternalInput")
    maskm_d = nc.dram_tensor("maskm", [N_NODES, ROWS], mybir.dt.float8e4,
                             kind="ExternalInput")
    wtaug_d = nc.dram_tensor("wt_aug", [IN_DIM, WTW], F16,
                             kind="ExternalInput")
    wc_d = nc.dram_tensor("wconst", [128, OUT_DIM + 3], F32,
                          kind="ExternalInput")
    out_d = nc.dram_tensor("out", [ROWS, OUT_DIM], F32, kind="ExternalOutput")

    with tile.TileContext(nc) as tc:
        with (
            tc.tile_pool(name="consts", bufs=1) as consts,
            tc.tile_pool(name="ndpool", bufs=3) as ndpool,
            tc.tile_pool(name="grpp", bufs=4) as grpp,
            tc.tile_pool(name="outp", bufs=2) as outp,
        ):
            # ---- persistent tiles ----
            wh_aug = consts.tile([128, NCK, WAUG], F16)
            e128 = consts.tile([128, NCK], F32)
            r128 = consts.tile([128, NCK], F32)
            p_row = consts.tile([1, ROWS], F16)
            p_b = consts.tile([128, ROWS], F16)
            warm_a = consts.tile([128, 64], F16)
            nc.gpsimd.memset(warm_a[:], 0.0)

            # ---- block-0/1 node DMAs first (critical path), 2 issues each
            ndT_tiles = {}

            def dma_ndT(b):
                ndT = ndpool.tile([128, 4, ROWS], F16, name="ndT", tag="ndT")
                for half, eng in ((0, nc.sync), (1, nc.scalar)):
                    src = nodesT_d[half * 256:(half + 1) * 256,
                                   b * ROWS:(b + 1) * ROWS]
                    src = src.rearrange("(d p) n -> p d n", p=128)
                    eng.dma_start(ndT[:, half * 2:half * 2 + 2, :], src)
                ndT_tiles[b] = ndT

            dma_ndT(0)

            # weights (scalar engine queue): col 258 of wt_aug = v1
            wt_t = []
            for d4 in range(4):
                w = consts.tile([128, WTW], F16, name=f"wt{d4}",
                                tag=f"wt{d4}")
                nc.scalar.dma_start(w[:], wtaug_d[d4 * 128:(d4 + 1) * 128, :])
                wt_t.append(w)
            # wconst: cols 0:256 = W_b broadcast, 256:259 = cb consts
            wconst = consts.tile([128, OUT_DIM + 3], F32)
            nc.sync.dma_start(wconst[:], wc_d[:])
            wb_bc = wconst[:, 0:OUT_DIM]
            cb = wconst[:, OUT_DIM:OUT_DIM + 3]

            dma_ndT(1)

            # ---- mask prefetch for the first two groups ----
            mask_tiles = {}

            def dma_mask(h, g, suffix="", bufs=4):
                mgrp = grpp.tile([128, GRP, HALF], F16, name=f"mgrp{suffix}",
                                 tag=f"mgrp{suffix}", bufs=bufs)
                msrc = maskm_d[g * GRP * 128:(g + 1) * GRP * 128,
                               h * HALF:(h + 1) * HALF]
                msrc = msrc.rearrange("(c p) i -> p c i", p=128)
                nc.gpsimd.dma_start(mgrp[:], msrc)  # fp8->fp16 cast
                mask_tiles[(h, g)] = mgrp
                return mgrp

            dma_mask(0, 0)
            dma_mask(0, 1)

            with (
                tc.tile_pool(name="psW", bufs=1, space="PSUM") as psW,
                tc.tile_pool(name="psH", bufs=3, space="PSUM") as psH,
                tc.tile_pool(name="psB", bufs=1, space="PSUM") as psB,
            ):
                # ---- PE warm-up: dummy matmuls on zeroed scratch ----
                warm_ps = psW.tile([1, HALF], F32, name="warm_ps", tag="pw1")
                for k in range(WARM_MMS):
                    nc.tensor.matmul(
                        warm_ps[:, 0:64], warm_a[:, 0:1], warm_a[:, 0:64],
                        start=(k == 0), stop=(k == WARM_MMS - 1),
                        skip_group_check=True)

                ndT0 = ndT_tiles[0]

                # ---- Wh1 row for own block; p = exp(0.8*Wh1) ----
                for h2 in range(2):
                    pw1 = psW.tile([1, HALF], F32, name="pw1", tag="pw1")
                    for d4 in range(4):
                        nc.tensor.matmul(
                            pw1[:], wt_t[d4][:, 258:259],
                            ndT0[:, d4, h2 * HALF:(h2 + 1) * HALF],
                            start=(d4 == 0), stop=(d4 == 3),
                            skip_group_check=True)
                    nc.scalar.activation(
                        p_row[:, h2 * HALF:(h2 + 1) * HALF], pw1[:], EXP,
                        scale=1.0 - ALPHA, bias=cb[0:1, 0:1])
                    nc.gpsimd.partition_broadcast(
                        p_b[:, h2 * HALF:(h2 + 1) * HALF],
                        p_row[:, h2 * HALF:(h2 + 1) * HALF])

                def build_block(b, dve_copy=False):
                    ndT = ndT_tiles.pop(b)
                    for ckl in range(8):
                        ck = b * 8 + ckl
                        pwh = psH.tile([128, WAUG], F32, name="pwh",
                                       tag="pwh", bufs=3)
                        for d4 in range(4):
                            nc.tensor.matmul(
                                pwh[:],
                                ndT[:, d4, ckl * 128:(ckl + 1) * 128],
                                wt_t[d4][:, 0:WAUG],
                                start=(d4 == 0), stop=(d4 == 3),
                                skip_group_check=True)
                        # col 257 of pwh = Wh2 (wt_aug col 257 = W.T@a2)
                        nc.scalar.activation(
                            e128[:, ck:ck + 1], pwh[:, 257:258], EXP,
                            scale=-(1.0 - ALPHA), bias=cb[:, 1:2])
                        nc.scalar.activation(
                            r128[:, ck:ck + 1], pwh[:, 257:258], EXP,
                            scale=1.0, bias=cb[:, 2:3])
                        if dve_copy or (ckl % 2 == 1):
                            nc.vector.tensor_scalar(
                                wh_aug[:, ck, :], pwh[:],
                                r128[:, ck:ck + 1], None, op0=MULT)
                        else:
                            nc.scalar.activation(
                                wh_aug[:, ck, :], pwh[:], COPY,
                                scale=r128[:, ck:ck + 1])
                    # denominator column = r_j (batched, strided fp16 write)
                    nc.vector.tensor_copy(
                        wh_aug[:, b * 8:b * 8 + 8, OUT_DIM:OUT_DIM + 1],
                        r128[:, b * 8:b * 8 + 8])

                build_block(0, dve_copy=True)
                dma_ndT(2)
                build_block(1, dve_copy=True)

                def emit_x(h, g, suffix="", bufs=4):
                    mgrp = mask_tiles.pop((h, g), None)
                    if mgrp is None:
                        mgrp = dma_mask(h, g, suffix, bufs)
                        mask_tiles.pop((h, g))
                    xgrp = grpp.tile([128, GRP, HALF], F16,
                                     name=f"xgrp{suffix}",
                                     tag=f"xgrp{suffix}", bufs=bufs)
                    for ckl in range(GRP):
                        ck = g * GRP + ckl
                        nc.vector.scalar_tensor_tensor(
                            xgrp[:, ckl, :],
                            p_b[:, h * HALF:(h + 1) * HALF],
                            e128[:, ck:ck + 1],
                            mgrp[:, ckl, :],
                            op0=MAX, op1=MULT)
                    return xgrp

                def readout(h, accs):
                    o4 = outp.tile([128, 4, OUT_DIM], F32, name="o4",
                                   tag="o4", bufs=2)
                    for ib in range(4):
                        recip = outp.tile([128, 1], F32, name="recip",
                                          tag="recip", bufs=4)
                        nc.vector.reciprocal(
                            recip[:], accs[ib][:, OUT_DIM:OUT_DIM + 1])
                        nc.vector.scalar_tensor_tensor(
                            o4[:, ib, :], accs[ib][:, 0:OUT_DIM], recip[:],
                            wb_bc, op0=MULT, op1=ADD)
                        if ib % 2 == 1:
                            r0 = h * HALF + (ib - 1) * 128
                            dst = out_d[r0:r0 + 256, :]
                            dst = dst.rearrange("(b p) k -> p b k", p=128)
                            eng = nc.sync if ib == 1 else nc.scalar
                            eng.dma_start(dst, o4[:, ib - 1:ib + 1, :])

                # build schedule: blocks 0-2 handled above; block b (3..7)
                # DMA'd at group b-3, built at group b-2.
                prefetched = []
                for h in range(2):
                    accs = [psB.tile([128, WAUG], F32, name=f"acc{ib}",
                                     tag=f"acc{ib}") for ib in range(4)]
                    for g in range(NG):
                        if h == 0 and g < 6:
                            if g + 3 <= 7:
                                dma_ndT(g + 3)
                            build_block(g + 2)
                        if h == 1 and g < len(prefetched):
                            xgrp = prefetched[g]
                        else:
                            xgrp = emit_x(h, g)
                        last_g = (g == NG - 1)
                        if last_g:
                            # ib-outer so acc banks retire in order
                            for ib in range(4):
                                for ckl in range(GRP):
                                    ck = g * GRP + ckl
                                    nc.tensor.matmul(
                                        accs[ib][:],
                                        xgrp[:, ckl, ib * 128:(ib + 1) * 128],
                                        wh_aug[:, ck, :],
                                        start=False, stop=(ckl == GRP - 1),
                                        skip_group_check=True)
                        else:
                            for ckl in range(GRP):
                                ck = g * GRP + ckl
                                for ib in range(4):
                                    nc.tensor.matmul(
                                        accs[ib][:],
                                        xgrp[:, ckl, ib * 128:(ib + 1) * 128],
                                        wh_aug[:, ck, :],
                                        start=(ck == 0), stop=False,
                                        skip_group_check=True)
                    if h == 0:
                        # build h=1's first two score groups while the DVE
                        # is idle at the tail of h=0
                        prefetched = [emit_x(1, 0, "p", 2),
                                      emit_x(1, 1, "p", 2)]
                    readout(h, accs)
    nc.compile()
    return nc


def _get_nc():
    if "nc" not in _CACHE:
        _CACHE["nc"] = _build_nc()
    return _CACHE["nc"]


def _prep_in_maps(nodes, edge_mat, W_w, W_b, a1_w, a1_b, a2_w, a2_b):
    f16 = np.float16
    nodes = np.asarray(nodes, dtype=np.float32)
    edge_mat = np.asarray(edge_mat, dtype=bool)
    W_w = np.asarray(W_w, dtype=np.float32)
    W_b = np.asarray(W_b, dtype=np.float32)
    a1_w = np.asarray(a1_w, dtype=np.float32)
    a1_b = np.asarray(a1_b, dtype=np.float32)
    a2_w = np.asarray(a2_w, dtype=np.float32)
    a2_b = np.asarray(a2_b, dtype=np.float32)

    nodesT = np.ascontiguousarray(nodes.T).astype(f16)          # [512, 8192]
    v1 = (W_w.T @ a1_w[0]).astype(f16)[:, None]                 # [512, 1]
    v2 = (W_w.T @ a2_w[0]).astype(f16)[:, None]
    zc = np.zeros((IN_DIM, 1), f16)
    # cols: 0:256 = W.T, 256 = 0 (denom slot), 257 = v2, 258 = v1, 259 = 0
    wt_aug = np.concatenate([W_w.T.astype(f16), zc, v2, v1, zc], axis=1)
    c1v = float(W_b @ a1_w[0]) + float(a1_b[0])
    c2v = float(W_b @ a2_w[0]) + float(a2_b[0])
    # wconst cols 0:256 = W_b bcast; 256 = p bias; 257 = e bias; 258 = r bias
    wconst = np.concatenate([
        np.broadcast_to(W_b[None, :], (128, OUT_DIM)),
        np.broadcast_to(np.array(
            [(1.0 - ALPHA) * c1v, -(1.0 - ALPHA) * c2v, c2v],
            np.float32)[None, :], (128, 3)),
    ], axis=1).astype(np.float32)
    # multiplicative {0,1} mask, transposed, fp8 (cast to fp16 during DMA)
    import ml_dtypes
    maskT = np.where(edge_mat, 1, 0).astype(ml_dtypes.float8_e4m3fn).T

    in_maps = []
    for c in range(N_CORES):
        rs = c * ROWS
        sl = slice(rs, rs + ROWS)
        # roll node columns / mask rows so each core's own block is first
        nodesT_c = np.ascontiguousarray(
            np.concatenate([nodesT[:, rs:], nodesT[:, :rs]], axis=1))
        maskm_c = np.ascontiguousarray(
            np.concatenate([maskT[rs:, sl], maskT[:rs, sl]], axis=0))
        in_maps.append({
            "nodesT": nodesT_c,
            "maskm": maskm_c,
            "wt_aug": wt_aug,
            "wconst": wconst,
        })
    return in_maps


def _run(inputs, trace=False, trace_cores=None):
    from concourse.bass_utils import run_bass_kernel_spmd
    if trace:
        _ensure_ntff_hook()
    nc = _get_nc()
    in_maps = _prep_in_maps(**inputs)
    res = run_bass_kernel_spmd(nc, in_maps, list(range(N_CORES)),
                               trace=trace, trace_cores=trace_cores)
    out = np.concatenate([res.results[c]["out"] for c in range(N_CORES)],
                         axis=0)
    return out, res


def kernel(**inputs) -> np.ndarray:
    out, _ = _run(inputs, trace=False)
    return out


# revision 5
# speedup vs baseline: 1.0203x; 1.0035x over previous
"""DenseGraphAttentionHead Trainium2 Bass kernel (8-core SPMD row-sharded).

reference math:
    Wh = nodes @ W_w.T + W_b                    [N, 256]
    Wh1 = Wh @ a1_w.T + a1_b                    [N, 1]
    Wh2 = Wh @ a2_w.T + a2_b                    [N, 1]
    scores = leaky_relu(Wh1 + Wh2.T, 0.2)       [N, N]
    attention = softmax(where(edge, scores, -inf), axis=1)
    out = attention @ Wh                        [N, 256]

Key identity: softmax over j is invariant to per-row(i) factors, so with
    p[i] = exp(0.8*Wh1[i]),  q[j] = exp(0.2*Wh2[j]),  r[j] = exp(Wh2[j])
we have  exp(lrelu(Wh1+Wh2) - 0.2*Wh1) = max(q[j], r[j]*p[i])
(branch r*p >= q  <=>  Wh1+Wh2 >= 0, the lrelu branch), hence
    attention_ij ∝ edge_ij * max(q[j], r[j]*p[i]).
The dense exp/lrelu over the 8192x8192 score matrix collapses to one fused
DVE tensor_scalar (mult+max) per 128x512 tile plus one group-batched
tensor_tensor multiply with the {0,1} edge mask (fp8 in HBM, upcast during
the SWDGE DMA); exps only run on small vectors.

Per core c (rows i in [c*1024, (c+1)*1024)):
  nodesT is column-rolled per core so chunk order starts at the core's own
  block; mask rows are rolled identically, so SPMD code is rank-oblivious.
  psum[i, 0:258] += X[:, i_blk].T @ wh_aug over j chunks; col 256 = softmax
  denominator (ones column), col 257 dead rider. out = psum[:, :256]/denom
  + W_b (softmax rows sum to 1, so +W_b commutes with attention@).
"""
import sys
import types

import numpy as np

N_NODES = 8192
IN_DIM = 512
OUT_DIM = 256
ALPHA = 0.2
N_CORES = 8
ROWS = N_NODES // N_CORES          # 1024 rows per core
NCK = N_NODES // 128               # 64 j-chunks of 128
NBLK = 8                           # j-blocks of 8 chunks (1024 nodes)
GRP = 4                            # j-chunks per mask-DMA batch
HALF = 512
NG = NCK // GRP                    # 16 groups of GRP chunks per i-half
WAUG = OUT_DIM + 2                 # 256 value cols + denom col + rider
WTW = OUT_DIM + 4                  # wt cols: 256 W + 0 + v2 + v1 + pad
WARM_MMS = 72                      # PE warm-up dummy matmuls (~4us)
GPS_TT_GROUPS = 6                  # h0 groups whose mask-mult runs on gpsimd

_CACHE = {}


def _ensure_ntff_hook():
    """antenv.axon_hooks is absent in this container; shim it so
    run_bass_kernel_spmd(trace=True) can reach the NTFF profiler."""
    if "antenv.axon_hooks" in sys.modules:
        return
    holder = [None]
    mod = types.ModuleType("antenv.axon_hooks")
    mod.set_axon_ntff_profile_hook = lambda h: holder.__setitem__(0, h)
    mod.get_axon_ntff_profile_hook = lambda: holder[0]
    sys.modules["antenv.axon_hooks"] = mod
    try:
        from trn_agent_boot.trn_boot import _ntff_profile_via_ctypes
        mod.set_axon_ntff_profile_hook(
            _ntff_profile_via_ctypes("/opt/axon/libaxon_pjrt.so"))
    except Exception:
        pass


def _build_nc():
    import concourse.bacc as bacc
    import concourse.tile as tile
    from concourse import mybir

    F16 = mybir.dt.float16
    F32 = mybir.dt.float32
    MULT = mybir.AluOpType.mult
    MAX = mybir.AluOpType.max
    ADD = mybir.AluOpType.add
    EXP = mybir.ActivationFunctionType.Exp

    nc = bacc.Bacc("TRN2", target_bir_lowering=False, debug=False,
                   num_devices=N_CORES)

    nodesT_d = nc.dram_tensor("nodesT", [IN_DIM, N_NODES], F16,
                              kind="ExternalInput")
    maskm_d = nc.dram_tensor("maskm", [N_NODES, ROWS], mybir.dt.float8e4,
                             kind="ExternalInput")
    wtaug_d = nc.dram_tensor("wt_aug", [IN_DIM, WTW], F16,
                             kind="ExternalInput")
    wc_d = nc.dram_tensor("wconst", [128, OUT_DIM + 3], F32,
                          kind="ExternalInput")
    out_d = nc.dram_tensor("out", [ROWS, OUT_DIM], F32, kind="ExternalOutput")

    with tile.TileContext(nc) as tc:
        with (
            tc.tile_pool(name="consts", bufs=1) as consts,
            tc.tile_pool(name="ndpool", bufs=3) as ndpool,
            tc.tile_pool(name="grpp", bufs=4) as grpp,
            tc.tile_pool(name="outp", bufs=2) as outp,
        ):
            # ---- persistent tiles ----
            wh_aug = consts.tile([128, NCK, WAUG], F16)
            wh2f32 = consts.tile([128, NCK], F32)
            q128 = consts.tile([128, NCK], F32)
            r128 = consts.tile([128, NCK], F32)
            p_row = consts.tile([1, ROWS], F16)
            p_b = consts.tile([128, ROWS], F16)
            warm_a = consts.tile([128, 64], F16)
            nc.gpsimd.memset(warm_a[:], 0.0)
            # denominator column = 1, rider column = 0, for all chunks
            nc.gpsimd.memset(wh_aug[:, :, OUT_DIM:OUT_DIM + 1], 1.0)
            nc.gpsimd.memset(wh_aug[:, :, OUT_DIM + 1:OUT_DIM + 2], 0.0)

            # ---- block-0/1 node DMAs first (critical path), 2 issues each
            ndT_tiles = {}

            def dma_ndT(b):
                ndT = ndpool.tile([128, 4, ROWS], F16, name="ndT", tag="ndT")
                for half, eng in ((0, nc.sync), (1, nc.scalar)):
                    src = nodesT_d[half * 256:(half + 1) * 256,
                                   b * ROWS:(b + 1) * ROWS]
                    src = src.rearrange("(d p) n -> p d n", p=128)
                    eng.dma_start(ndT[:, half * 2:half * 2 + 2, :], src)
                ndT_tiles[b] = ndT

            dma_ndT(0)

            # weights (scalar engine queue): col 258 of wt_aug = v1
            wt_t = []
            for d4 in range(4):
                w = consts.tile([128, WTW], F16, name=f"wt{d4}",
                                tag=f"wt{d4}")
                nc.scalar.dma_start(w[:], wtaug_d[d4 * 128:(d4 + 1) * 128, :])
                wt_t.append(w)
            # wconst: cols 0:256 = W_b broadcast, 256:259 = cb consts
            wconst = consts.tile([128, OUT_DIM + 3], F32)
            nc.sync.dma_start(wconst[:], wc_d[:])
            wb_bc = wconst[:, 0:OUT_DIM]
            cb = wconst[:, OUT_DIM:OUT_DIM + 3]

            dma_ndT(1)

            # ---- mask DMA helper + prefetch of the first two groups ----
            mask_tiles = {}

            def dma_mask(h, g, suffix="", bufs=4):
                mgrp = grpp.tile([128, GRP, HALF], F16, name=f"mgrp{suffix}",
                                 tag=f"mgrp{suffix}", bufs=bufs)
                msrc = maskm_d[g * GRP * 128:(g + 1) * GRP * 128,
                               h * HALF:(h + 1) * HALF]
                msrc = msrc.rearrange("(c p) i -> p c i", p=128)
                nc.gpsimd.dma_start(mgrp[:], msrc)  # fp8->fp16 cast
                mask_tiles[(h, g)] = mgrp
                return mgrp

            dma_mask(0, 0)
            dma_mask(0, 1)

            with (
                tc.tile_pool(name="psW", bufs=1, space="PSUM") as psW,
                tc.tile_pool(name="psH", bufs=3, space="PSUM") as psH,
                tc.tile_pool(name="psB", bufs=1, space="PSUM") as psB,
            ):
                # ---- PE warm-up: dummy matmuls on zeroed scratch ----
                warm_ps = psW.tile([1, HALF], F32, name="warm_ps", tag="pw1")
                for k in range(WARM_MMS):
                    nc.tensor.matmul(
                        warm_ps[:, 0:64], warm_a[:, 0:1], warm_a[:, 0:64],
                        start=(k == 0), stop=(k == WARM_MMS - 1),
                        skip_group_check=True)

                ndT0 = ndT_tiles[0]

                # ---- Wh1 row for own block; p = exp(0.8*Wh1) ----
                for h2 in range(2):
                    pw1 = psW.tile([1, HALF], F32, name="pw1", tag="pw1")
                    for d4 in range(4):
                        nc.tensor.matmul(
                            pw1[:], wt_t[d4][:, 258:259],
                            ndT0[:, d4, h2 * HALF:(h2 + 1) * HALF],
                            start=(d4 == 0), stop=(d4 == 3),
                            skip_group_check=True)
                    nc.scalar.activation(
                        p_row[:, h2 * HALF:(h2 + 1) * HALF], pw1[:], EXP,
                        scale=1.0 - ALPHA, bias=cb[0:1, 0:1])
                    nc.gpsimd.partition_broadcast(
                        p_b[:, h2 * HALF:(h2 + 1) * HALF],
                        p_row[:, h2 * HALF:(h2 + 1) * HALF])

                def build_block(b, early=False):
                    ndT = ndT_tiles.pop(b)
                    for ckl in range(8):
                        ck = b * 8 + ckl
                        pwh = psH.tile([128, WAUG], F32, name="pwh",
                                       tag="pwh", bufs=3)
                        for d4 in range(4):
                            nc.tensor.matmul(
                                pwh[:],
                                ndT[:, d4, ckl * 128:(ckl + 1) * 128],
                                wt_t[d4][:, 0:WAUG],
                                start=(d4 == 0), stop=(d4 == 3),
                                skip_group_check=True)
                        # col 257 of pwh = Wh2 (wt_aug col 257 = W.T@a2)
                        if early:
                            nc.vector.tensor_copy(wh2f32[:, ck:ck + 1],
                                                  pwh[:, 257:258])
                        else:
                            nc.scalar.copy(wh2f32[:, ck:ck + 1],
                                           pwh[:, 257:258])
                        # value columns, plain fp32->fp16 copy
                        nc.scalar.copy(wh_aug[:, ck, 0:OUT_DIM],
                                       pwh[:, 0:OUT_DIM])
                    sl = slice(b * 8, b * 8 + 8)
                    nc.scalar.activation(q128[:, sl], wh2f32[:, sl], EXP,
                                         scale=ALPHA, bias=cb[:, 1:2])
                    nc.scalar.activation(r128[:, sl], wh2f32[:, sl], EXP,
                                         scale=1.0, bias=cb[:, 2:3])

                build_block(0, early=True)
                dma_ndT(2)
                build_block(1, early=True)

                def emit_x(h, g, suffix="", bufs=4, gps_tt=False):
                    mgrp = mask_tiles.pop((h, g), None)
                    if mgrp is None:
                        mgrp = dma_mask(h, g, suffix, bufs)
                        mask_tiles.pop((h, g))
                    sgrp = grpp.tile([128, GRP, HALF], F16,
                                     name=f"sgrp{suffix}",
                                     tag=f"sgrp{suffix}", bufs=bufs)
                    for ckl in range(GRP):
                        ck = g * GRP + ckl
                        nc.vector.tensor_scalar(
                            sgrp[:, ckl, :],
                            p_b[:, h * HALF:(h + 1) * HALF],
                            r128[:, ck:ck + 1], q128[:, ck:ck + 1],
                            op0=MULT, op1=MAX)
                    xgrp = grpp.tile([128, GRP, HALF], F16,
                                     name=f"xgrp{suffix}",
                                     tag=f"xgrp{suffix}",
                                     bufs=max(2, bufs - 1))
                    eng = nc.gpsimd if gps_tt else nc.vector
                    eng.tensor_tensor(xgrp[:], sgrp[:], mgrp[:], op=MULT)
                    return xgrp

                def readout(h, accs):
                    o4 = outp.tile([128, 4, OUT_DIM], F32, name="o4",
                                   tag="o4", bufs=2)
                    for ib in range(4):
                        recip = outp.tile([128, 1], F32, name="recip",
                                          tag="recip", bufs=4)
                        nc.vector.reciprocal(
                            recip[:], accs[ib][:, OUT_DIM:OUT_DIM + 1])
                        nc.vector.scalar_tensor_tensor(
                            o4[:, ib, :], accs[ib][:, 0:OUT_DIM], recip[:],
                            wb_bc, op0=MULT, op1=ADD)
                        if ib % 2 == 1:
                            r0 = h * HALF + (ib - 1) * 128
                            dst = out_d[r0:r0 + 256, :]
                            dst = dst.rearrange("(b p) k -> p b k", p=128)
                            eng = nc.sync if ib == 1 else nc.scalar
                            eng.dma_start(dst, o4[:, ib - 1:ib + 1, :])

                # build schedule: blocks 0-2 handled above; block b (3..7)
                # DMA'd at group b-3, built at group b-2.
                prefetched = []
                for h in range(2):
                    accs = [psB.tile([128, WAUG], F32, name=f"acc{ib}",
                                     tag=f"acc{ib}") for ib in range(4)]
                    for g in range(NG):
                        if (h, g + 2) not in mask_tiles and g + 2 < NG:
                            dma_mask(h, g + 2)
                        if h == 0 and g < 6:
                            if g + 3 <= 7:
                                dma_ndT(g + 3)
                            build_block(g + 2)
                        if h == 1 and g < len(prefetched):
                            xgrp = prefetched[g]
                        else:
                            xgrp = emit_x(
                                h, g,
                                gps_tt=(h == 0 and g < GPS_TT_GROUPS))
                        last_g = (g == NG - 1)
                        if last_g:
                            # ib-outer so acc banks retire in order
                            for ib in range(4):
                                for ckl in range(GRP):
                                    ck = g * GRP + ckl
                                    nc.tensor.matmul(
                                        accs[ib][:],
                                        xgrp[:, ckl, ib * 128:(ib + 1) * 128],
                                        wh_aug[:, ck, :],
                                        start=False, stop=(ckl == GRP - 1),
                                        skip_group_check=True)
                        else:
                            for ckl in range(GRP):
                                ck = g * GRP + ckl
                                for ib in range(4):
                                    nc.tensor.matmul(
                                        accs[ib][:],
                                        xgrp[:, ckl, ib * 128:(ib + 1) * 128],
                                        wh_aug[:, ck, :],
                                        start=(ck == 0), stop=False,
                                        skip_group_check=True)
                    if h == 0:
                        # build h=1's first two score groups while the DVE
                        # is idle at the tail of h=0
                        prefetched = [emit_x(1, 0, "p", 2),
                                      emit_x(1, 1, "p", 2)]
                    readout(h, accs)
    nc.compile()
    return nc


def _get_nc():
    if "nc" not in _CACHE:
        _CACHE["nc"] = _build_nc()
    return _CACHE["nc"]


def _prep_in_maps(nodes, edge_mat, W_w, W_b, a1_w, a1_b, a2_w, a2_b):
    f16 = np.float16
    nodes = np.asarray(nodes, dtype=np.float32)
    edge_mat = np.asarray(edge_mat, dtype=bool)
    W_w = np.asarray(W_w, dtype=np.float32)
    W_b = np.asarray(W_b, dtype=np.float32)
    a1_w = np.asarray(a1_w, dtype=np.float32)
    a1_b = np.asarray(a1_b, dtype=np.float32)
    a2_w = np.asarray(a2_w, dtype=np.float32)
    a2_b = np.asarray(a2_b, dtype=np.float32)

    nodesT = np.ascontiguousarray(nodes.T).astype(f16)          # [512, 8192]
    v1 = (W_w.T @ a1_w[0]).astype(f16)[:, None]                 # [512, 1]
    v2 = (W_w.T @ a2_w[0]).astype(f16)[:, None]
    zc = np.zeros((IN_DIM, 1), f16)
    # cols: 0:256 = W.T, 256 = 0 (denom slot), 257 = v2, 258 = v1, 259 = 0
    wt_aug = np.concatenate([W_w.T.astype(f16), zc, v2, v1, zc], axis=1)
    c1v = float(W_b @ a1_w[0]) + float(a1_b[0])
    c2v = float(W_b @ a2_w[0]) + float(a2_b[0])
    # wconst cols 0:256 = W_b bcast; 256 = p bias; 257 = q bias; 258 = r bias
    wconst = np.concatenate([
        np.broadcast_to(W_b[None, :], (128, OUT_DIM)),
        np.broadcast_to(np.array(
            [(1.0 - ALPHA) * c1v, ALPHA * c2v, c2v],
            np.float32)[None, :], (128, 3)),
    ], axis=1).astype(np.float32)
    # multiplicative {0,1} mask, transposed, fp8 (cast to fp16 during DMA)
    import ml_dtypes
    maskT = np.where(edge_mat, 1, 0).astype(ml_dtypes.float8_e4m3fn).T

    in_maps = []
    for c in range(N_CORES):
        rs = c * ROWS
        sl = slice(rs, rs + ROWS)
        # roll node columns / mask rows so each core's own block is first
        nodesT_c = np.ascontiguousarray(
            np.concatenate([nodesT[:, rs:], nodesT[:, :rs]], axis=1))
        maskm_c = np.ascontiguousarray(
            np.concatenate([maskT[rs:, sl], maskT[:rs, sl]], axis=0))
        in_maps.append({
            "nodesT": nodesT_c,
            "maskm": maskm_c,
            "wt_aug": wt_aug,
            "wconst": wconst,
        })
    return in_maps


def _run(inputs, trace=False, trace_cores=None):
    from concourse.bass_utils import run_bass_kernel_spmd
    if trace:
        _ensure_ntff_hook()
    nc = _get_nc()
    in_maps = _prep_in_maps(**inputs)
    res = run_bass_kernel_spmd(nc, in_maps, list(range(N_CORES)),
                               trace=trace, trace_cores=trace_cores)
    out = np.concatenate([res.results[c]["out"] for c in range(N_CORES)],
                         axis=0)
    return out, res


def kernel(**inputs) -> np.ndarray:
    out, _ = _run(inputs, trace=False)
    return out


# revision 13
# speedup vs baseline: 1.1432x; 1.1204x over previous
"""DenseGraphAttentionHead Trainium2 Bass kernel (8-core SPMD row-sharded).

reference math:
    Wh = nodes @ W_w.T + W_b                    [N, 256]
    Wh1 = Wh @ a1_w.T + a1_b                    [N, 1]
    Wh2 = Wh @ a2_w.T + a2_b                    [N, 1]
    scores = leaky_relu(Wh1 + Wh2.T, 0.2)       [N, N]
    attention = softmax(where(edge, scores, -inf), axis=1)
    out = attention @ Wh                        [N, 256]

Key identity: softmax over j is invariant to per-row(i) factors, so with
    p[i] = exp(0.8*Wh1[i]),  q[j] = exp(0.2*Wh2[j]),  r[j] = exp(Wh2[j])
we have  exp(lrelu(Wh1+Wh2) - 0.2*Wh1) = max(q[j], r[j]*p[i])
(branch r*p >= q  <=>  Wh1+Wh2 >= 0, the lrelu branch), hence
    attention_ij ∝ edge_ij * max(q[j], r[j]*p[i]).
The dense exp/lrelu over the 8192x8192 score matrix collapses to one fused
DVE tensor_scalar (mult+max) per 128x512 tile plus one group-batched
tensor_tensor multiply with the {0,1} edge mask (fp8 in HBM, upcast during
the SWDGE DMA); exps only run on small vectors.

Per core c (rows i in [c*1024, (c+1)*1024)):
  nodesT is column-rolled per core so chunk order starts at the core's own
  block; mask rows are rolled identically, so SPMD code is rank-oblivious.
  psum[i, 0:258] += X[:, i_blk].T @ wh_aug over j chunks; col 256 = softmax
  denominator (ones column), col 257 dead rider. out = psum[:, :256]/denom
  + W_b (softmax rows sum to 1, so +W_b commutes with attention@).
"""
import sys
import types

import numpy as np

N_NODES = 8192
IN_DIM = 512
OUT_DIM = 256
ALPHA = 0.2
N_CORES = 8
ROWS = N_NODES // N_CORES          # 1024 rows per core
NCK = N_NODES // 128               # 64 j-chunks of 128
NBLK = 8                           # j-blocks of 8 chunks (1024 nodes)
GRP = 4                            # j-chunks per mask-DMA batch
HALF = 512
NG = NCK // GRP                    # 16 groups of GRP chunks per i-half
WAUG = OUT_DIM + 2                 # 256 value cols + denom col + rider
WTW = OUT_DIM + 4                  # wt cols: 256 W + 0 + v2 + v1 + pad
WARM_MMS = 96                      # PE warm-up dummy matmuls (~4.6us)

_CACHE = {}


def _ensure_ntff_hook():
    """antenv.axon_hooks is absent in this container; shim it so
    run_bass_kernel_spmd(trace=True) can reach the NTFF profiler."""
    if "antenv.axon_hooks" in sys.modules:
        return
    holder = [None]
    mod = types.ModuleType("antenv.axon_hooks")
    mod.set_axon_ntff_profile_hook = lambda h: holder.__setitem__(0, h)
    mod.get_axon_ntff_profile_hook = lambda: holder[0]
    sys.modules["antenv.axon_hooks"] = mod
    try:
        from trn_agent_boot.trn_boot import _ntff_profile_via_ctypes
        mod.set_axon_ntff_profile_hook(
            _ntff_profile_via_ctypes("/opt/axon/libaxon_pjrt.so"))
    except Exception:
        pass


def _build_nc():
    import concourse.bacc as bacc
    import concourse.tile as tile
    from concourse import mybir

    F16 = mybir.dt.float16
    F32 = mybir.dt.float32
    MULT = mybir.AluOpType.mult
    MAX = mybir.AluOpType.max
    ADD = mybir.AluOpType.add
    EXP = mybir.ActivationFunctionType.Exp

    nc = bacc.Bacc("TRN2", target_bir_lowering=False, debug=False,
                   num_devices=N_CORES)

    nodesT_d = nc.dram_tensor("nodesT", [IN_DIM, N_NODES], F16,
                              kind="ExternalInput")
    maskm_d = nc.dram_tensor("maskm", [N_NODES, ROWS], mybir.dt.float8e4,
                             kind="ExternalInput")
    wtaug_d = nc.dram_tensor("wt_aug", [IN_DIM, WTW], F16,
                             kind="ExternalInput")
    v1_d = nc.dram_tensor("v1t", [128, 4], F16, kind="ExternalInput")
    wc_d = nc.dram_tensor("wconst", [128, OUT_DIM + 3], F32,
                          kind="ExternalInput")
    out_d = nc.dram_tensor("out", [ROWS, OUT_DIM], F32, kind="ExternalOutput")

    with tile.TileContext(nc) as tc:
        with (
            tc.tile_pool(name="consts", bufs=1) as consts,
            tc.tile_pool(name="ndpool", bufs=3) as ndpool,
            tc.tile_pool(name="grpp", bufs=4) as grpp,
            tc.tile_pool(name="outp", bufs=2) as outp,
        ):
            # ---- persistent tiles ----
            wh_aug = consts.tile([128, NCK, WAUG], F16)
            wh2f32 = consts.tile([128, NCK], F32)
            q128 = consts.tile([128, NCK], F32)
            r128 = consts.tile([128, NCK], F32)
            p_row = consts.tile([1, ROWS], F16)
            p_b = consts.tile([128, ROWS], F16)
            warm_a = consts.tile([128, 64], F16)
            nc.gpsimd.memset(warm_a[:], 0.0)
            # denominator column = 1, rider column = 0, for all chunks
            nc.gpsimd.memset(wh_aug[:, :, OUT_DIM:OUT_DIM + 1], 1.0)
            nc.gpsimd.memset(wh_aug[:, :, OUT_DIM + 1:OUT_DIM + 2], 0.0)

            # ---- critical-path DMAs first.  pw1 depends only on sync-issued
            # data (v1 + both ndT0 halves); builds need wt (scalar, first).
            ndT_tiles = {}

            def dma_ndT(b, engines=(None, None)):
                e0 = engines[0] or nc.sync
                e1 = engines[1] or nc.scalar
                ndT = ndpool.tile([128, 4, ROWS], F16, name="ndT", tag="ndT")
                for half, eng in ((0, e0), (1, e1)):
                    src = nodesT_d[half * 256:(half + 1) * 256,
                                   b * ROWS:(b + 1) * ROWS]
                    src = src.rearrange("(d p) n -> p d n", p=128)
                    eng.dma_start(ndT[:, half * 2:half * 2 + 2, :], src)
                ndT_tiles[b] = ndT

            v1t = consts.tile([128, 4], F16)
            nc.sync.dma_start(v1t[:], v1_d[:])
            wt_t = []
            for d4 in range(4):
                w = consts.tile([128, WTW], F16, name=f"wt{d4}",
                                tag=f"wt{d4}")
                nc.scalar.dma_start(w[:], wtaug_d[d4 * 128:(d4 + 1) * 128, :])
                wt_t.append(w)
            dma_ndT(0, engines=(nc.sync, nc.sync))
            # wconst: cols 0:256 = W_b broadcast, 256:259 = cb consts
            wconst = consts.tile([128, OUT_DIM + 3], F32)
            nc.sync.dma_start(wconst[:], wc_d[:])
            wb_bc = wconst[:, 0:OUT_DIM]
            cb = wconst[:, OUT_DIM:OUT_DIM + 3]

            dma_ndT(1)

            # ---- mask DMA helper + prefetch of the first two groups ----
            mask_tiles = {}

            def dma_mask(h, g, suffix="", bufs=4):
                mgrp = grpp.tile([128, GRP, HALF], F16, name=f"mgrp{suffix}",
                                 tag=f"mgrp{suffix}", bufs=bufs)
                msrc = maskm_d[g * GRP * 128:(g + 1) * GRP * 128,
                               h * HALF:(h + 1) * HALF]
                msrc = msrc.rearrange("(c p) i -> p c i", p=128)
                nc.gpsimd.dma_start(mgrp[:], msrc)  # fp8->fp16 cast
                mask_tiles[(h, g)] = mgrp
                return mgrp

            dma_mask(0, 0)
            dma_mask(0, 1)

            with (
                tc.tile_pool(name="psW", bufs=1, space="PSUM") as psW,
                tc.tile_pool(name="psH", bufs=3, space="PSUM") as psH,
                tc.tile_pool(name="psB", bufs=1, space="PSUM") as psB,
            ):
                # ---- PE warm-up: dummy matmuls on zeroed scratch ----
                warm_ps = psW.tile([1, HALF], F32, name="warm_ps", tag="pw1")
                for k in range(WARM_MMS):
                    nc.tensor.matmul(
                        warm_ps[:, 0:64], warm_a[:, 0:1], warm_a[:, 0:64],
                        start=(k == 0), stop=(k == WARM_MMS - 1),
                        skip_group_check=True)

                ndT0 = ndT_tiles[0]

                # ---- Wh1 row for own block; p = exp(0.8*Wh1) ----
                for h2 in range(2):
                    pw1 = psW.tile([1, HALF], F32, name="pw1", tag="pw1")
                    for d4 in range(4):
                        nc.tensor.matmul(
                            pw1[:], v1t[:, d4:d4 + 1],
                            ndT0[:, d4, h2 * HALF:(h2 + 1) * HALF],
                            start=(d4 == 0), stop=(d4 == 3),
                            skip_group_check=True)
                    nc.scalar.activation(
                        p_row[:, h2 * HALF:(h2 + 1) * HALF], pw1[:], EXP,
                        scale=1.0 - ALPHA, bias=cb[0:1, 0:1])
                    nc.gpsimd.partition_broadcast(
                        p_b[:, h2 * HALF:(h2 + 1) * HALF],
                        p_row[:, h2 * HALF:(h2 + 1) * HALF])

                def build_block(b, early=False):
                    ndT = ndT_tiles.pop(b)
                    for ckl in range(8):
                        ck = b * 8 + ckl
                        pwh = psH.tile([128, WAUG], F32, name="pwh",
                                       tag="pwh", bufs=3)
                        for d4 in range(4):
                            nc.tensor.matmul(
                                pwh[:],
                                ndT[:, d4, ckl * 128:(ckl + 1) * 128],
                                wt_t[d4][:, 0:WAUG],
                                start=(d4 == 0), stop=(d4 == 3),
                                skip_group_check=True)
                        # col 257 of pwh = Wh2 (wt_aug col 257 = W.T@a2)
                        if early:
                            nc.vector.tensor_copy(wh2f32[:, ck:ck + 1],
                                                  pwh[:, 257:258])
                        else:
                            nc.scalar.copy(wh2f32[:, ck:ck + 1],
                                           pwh[:, 257:258])
                        # value columns, plain fp32->fp16 copy
                        nc.scalar.copy(wh_aug[:, ck, 0:OUT_DIM],
                                       pwh[:, 0:OUT_DIM])
                    sl = slice(b * 8, b * 8 + 8)
                    nc.scalar.activation(q128[:, sl], wh2f32[:, sl], EXP,
                                         scale=ALPHA, bias=cb[:, 1:2])
                    nc.scalar.activation(r128[:, sl], wh2f32[:, sl], EXP,
                                         scale=1.0, bias=cb[:, 2:3])

                build_block(0, early=True)
                dma_ndT(2)
                build_block(1, early=True)

                def emit_x(h, g, suffix="", bufs=4):
                    mgrp = mask_tiles.pop((h, g), None)
                    if mgrp is None:
                        mgrp = dma_mask(h, g, suffix, bufs)
                        mask_tiles.pop((h, g))
                    sgrp = grpp.tile([128, GRP, HALF], F16,
                                     name=f"sgrp{suffix}",
                                     tag=f"sgrp{suffix}", bufs=bufs)
                    for ckl in range(GRP):
                        ck = g * GRP + ckl
                        nc.vector.tensor_scalar(
                            sgrp[:, ckl, :],
                            p_b[:, h * HALF:(h + 1) * HALF],
                            r128[:, ck:ck + 1], q128[:, ck:ck + 1],
                            op0=MULT, op1=MAX)
                    xgrp = grpp.tile([128, GRP, HALF], F16,
                                     name=f"xgrp{suffix}",
                                     tag=f"xgrp{suffix}",
                                     bufs=max(2, bufs - 1))
                    nc.vector.tensor_tensor(xgrp[:], sgrp[:], mgrp[:],
                                            op=MULT)
                    return xgrp

                def readout_ib(h, accs, ib, o4):
                    recip = outp.tile([128, 1], F32, name="recip",
                                      tag="recip", bufs=4)
                    nc.vector.reciprocal(
                        recip[:], accs[ib][:, OUT_DIM:OUT_DIM + 1])
                    nc.vector.scalar_tensor_tensor(
                        o4[:, ib, :], accs[ib][:, 0:OUT_DIM], recip[:],
                        wb_bc, op0=MULT, op1=ADD)
                    if ib % 2 == 1:
                        r0 = h * HALF + (ib - 1) * 128
                        dst = out_d[r0:r0 + 256, :]
                        dst = dst.rearrange("(b p) k -> p b k", p=128)
                        eng = nc.sync if ib == 1 else nc.scalar
                        eng.dma_start(dst, o4[:, ib - 1:ib + 1, :])

                # build schedule: blocks 0-2 handled above; block b (3..7)
                # DMA'd at group b-3, built at group b-2.
                prefetched = []
                for h in range(2):
                    accs = [psB.tile([128, WAUG], F32, name=f"acc{ib}",
                                     tag=f"acc{ib}") for ib in range(4)]
                    for g in range(NG):
                        if (h, g + 2) not in mask_tiles and g + 2 < NG:
                            dma_mask(h, g + 2)
                        if h == 0 and g < 6:
                            if g + 3 <= 7:
                                dma_ndT(g + 3)
                            build_block(g + 2)
                        if h == 1 and g < len(prefetched):
                            xgrp = prefetched[g]
                        else:
                            xgrp = emit_x(h, g)
                        last_g = (g == NG - 1)
                        if last_g:
                            # ib-outer so acc banks retire in order
                            for ib in range(4):
                                for ckl in range(GRP):
                                    ck = g * GRP + ckl
                                    nc.tensor.matmul(
                                        accs[ib][:],
                                        xgrp[:, ckl, ib * 128:(ib + 1) * 128],
                                        wh_aug[:, ck, :],
                                        start=False, stop=(ckl == GRP - 1),
                                        skip_group_check=True)
                        else:
                            for ckl in range(GRP):
                                ck = g * GRP + ckl
                                for ib in range(4):
                                    nc.tensor.matmul(
                                        accs[ib][:],
                                        xgrp[:, ckl, ib * 128:(ib + 1) * 128],
                                        wh_aug[:, ck, :],
                                        start=(ck == 0), stop=False,
                                        skip_group_check=True)
                    o4 = outp.tile([128, 4, OUT_DIM], F32, name="o4",
                                   tag="o4", bufs=2)
                    if h == 0:
                        # build h=1's first two score groups around the h0
                        # readout so neither blocks the other on the DVE
                        prefetched = [emit_x(1, 0, "p", 2)]
                        readout_ib(0, accs, 0, o4)
                        readout_ib(0, accs, 1, o4)
                        prefetched.append(emit_x(1, 1, "p", 2))
                        readout_ib(0, accs, 2, o4)
                        readout_ib(0, accs, 3, o4)
                    else:
                        for ib in range(4):
                            readout_ib(1, accs, ib, o4)
    nc.compile()
    return nc


def _get_nc():
    if "nc" not in _CACHE:
        _CACHE["nc"] = _build_nc()
    return _CACHE["nc"]


def _prep_in_maps(nodes, edge_mat, W_w, W_b, a1_w, a1_b, a2_w, a2_b):
    f16 = np.float16
    nodes = np.asarray(nodes, dtype=np.float32)
    edge_mat = np.asarray(edge_mat, dtype=bool)
    W_w = np.asarray(W_w, dtype=np.float32)
    W_b = np.asarray(W_b, dtype=np.float32)
    a1_w = np.asarray(a1_w, dtype=np.float32)
    a1_b = np.asarray(a1_b, dtype=np.float32)
    a2_w = np.asarray(a2_w, dtype=np.float32)
    a2_b = np.asarray(a2_b, dtype=np.float32)

    nodesT = np.ascontiguousarray(nodes.T).astype(f16)          # [512, 8192]
    v1 = (W_w.T @ a1_w[0]).astype(f16)[:, None]                 # [512, 1]
    v2 = (W_w.T @ a2_w[0]).astype(f16)[:, None]
    zc = np.zeros((IN_DIM, 1), f16)
    # cols: 0:256 = W.T, 256 = 0 (denom slot), 257 = v2, 258 = v1, 259 = 0
    wt_aug = np.concatenate([W_w.T.astype(f16), zc, v2, v1, zc], axis=1)
    c1v = float(W_b @ a1_w[0]) + float(a1_b[0])
    c2v = float(W_b @ a2_w[0]) + float(a2_b[0])
    # wconst cols 0:256 = W_b bcast; 256 = p bias; 257 = q bias; 258 = r bias
    wconst = np.concatenate([
        np.broadcast_to(W_b[None, :], (128, OUT_DIM)),
        np.broadcast_to(np.array(
            [(1.0 - ALPHA) * c1v, ALPHA * c2v, c2v],
            np.float32)[None, :], (128, 3)),
    ], axis=1).astype(np.float32)
    # multiplicative {0,1} mask, transposed, fp8 (cast to fp16 during DMA)
    import ml_dtypes
    maskT = np.where(edge_mat, 1, 0).astype(ml_dtypes.float8_e4m3fn).T

    in_maps = []
    for c in range(N_CORES):
        rs = c * ROWS
        sl = slice(rs, rs + ROWS)
        # roll node columns / mask rows so each core's own block is first
        nodesT_c = np.ascontiguousarray(
            np.concatenate([nodesT[:, rs:], nodesT[:, :rs]], axis=1))
        maskm_c = np.ascontiguousarray(
            np.concatenate([maskT[rs:, sl], maskT[:rs, sl]], axis=0))
        in_maps.append({
            "nodesT": nodesT_c,
            "maskm": maskm_c,
            "wt_aug": wt_aug,
            "v1t": np.ascontiguousarray(v1[:, 0].reshape(4, 128).T),
            "wconst": wconst,
        })
    return in_maps


def _run(inputs, trace=False, trace_cores=None):
    from concourse.bass_utils import run_bass_kernel_spmd
    if trace:
        _ensure_ntff_hook()
    nc = _get_nc()
    in_maps = _prep_in_maps(**inputs)
    res = run_bass_kernel_spmd(nc, in_maps, list(range(N_CORES)),
                               trace=trace, trace_cores=trace_cores)
    out = np.concatenate([res.results[c]["out"] for c in range(N_CORES)],
                         axis=0)
    return out, res


def kernel(**inputs) -> np.ndarray:
    out, _ = _run(inputs, trace=False)
    return out


# revision 14
# speedup vs baseline: 1.1691x; 1.0227x over previous
"""DenseGraphAttentionHead Trainium2 Bass kernel (8-core SPMD row-sharded).

reference math:
    Wh = nodes @ W_w.T + W_b                    [N, 256]
    Wh1 = Wh @ a1_w.T + a1_b                    [N, 1]
    Wh2 = Wh @ a2_w.T + a2_b                    [N, 1]
    scores = leaky_relu(Wh1 + Wh2.T, 0.2)       [N, N]
    attention = softmax(where(edge, scores, -inf), axis=1)
    out = attention @ Wh                        [N, 256]

Key identity: softmax over j is invariant to per-row(i) factors, so with
    p[i] = exp(0.8*Wh1[i]),  q[j] = exp(0.2*Wh2[j]),  r[j] = exp(Wh2[j])
we have  exp(lrelu(Wh1+Wh2) - 0.2*Wh1) = max(q[j], r[j]*p[i])
(branch r*p >= q  <=>  Wh1+Wh2 >= 0, the lrelu branch), hence
    attention_ij ∝ edge_ij * max(q[j], r[j]*p[i]).
The dense exp/lrelu over the 8192x8192 score matrix collapses to one fused
DVE tensor_scalar (mult+max) per 128x512 tile plus one group-batched
tensor_tensor multiply with the {0,1} edge mask (fp8 in HBM, upcast during
the SWDGE DMA); exps only run on small vectors.

Per core c (rows i in [c*1024, (c+1)*1024)):
  nodesT is column-rolled per core so chunk order starts at the core's own
  block; mask rows are rolled identically, so SPMD code is rank-oblivious.
  psum[i, 0:258] += X[:, i_blk].T @ wh_aug over j chunks; col 256 = softmax
  denominator (ones column), col 257 dead rider. out = psum[:, :256]/denom
  + W_b (softmax rows sum to 1, so +W_b commutes with attention@).
"""
import sys
import types

import numpy as np

N_NODES = 8192
IN_DIM = 512
OUT_DIM = 256
ALPHA = 0.2
N_CORES = 8
ROWS = N_NODES // N_CORES          # 1024 rows per core
NCK = N_NODES // 128               # 64 j-chunks of 128
NBLK = 8                           # j-blocks of 8 chunks (1024 nodes)
GRP = 4                            # j-chunks per mask-DMA batch
HALF = 512
NG = NCK // GRP                    # 16 groups of GRP chunks per i-half
WAUG = OUT_DIM + 2                 # 256 value cols + denom col + rider
WTW = OUT_DIM + 4                  # wt cols: 256 W + 0 + v2 + v1 + pad
WARM_MMS = 96                      # PE warm-up dummy matmuls (~4.6us)

_CACHE = {}


def _ensure_ntff_hook():
    """antenv.axon_hooks is absent in this container; shim it so
    run_bass_kernel_spmd(trace=True) can reach the NTFF profiler."""
    if "antenv.axon_hooks" in sys.modules:
        return
    holder = [None]
    mod = types.ModuleType("antenv.axon_hooks")
    mod.set_axon_ntff_profile_hook = lambda h: holder.__setitem__(0, h)
    mod.get_axon_ntff_profile_hook = lambda: holder[0]
    sys.modules["antenv.axon_hooks"] = mod
    try:
        from trn_agent_boot.trn_boot import _ntff_profile_via_ctypes
        mod.set_axon_ntff_profile_hook(
            _ntff_profile_via_ctypes("/opt/axon/libaxon_pjrt.so"))
    except Exception:
        pass


def _build_nc():
    import concourse.bacc as bacc
    import concourse.tile as tile
    from concourse import mybir

    F16 = mybir.dt.float16
    F32 = mybir.dt.float32
    MULT = mybir.AluOpType.mult
    MAX = mybir.AluOpType.max
    ADD = mybir.AluOpType.add
    EXP = mybir.ActivationFunctionType.Exp

    nc = bacc.Bacc("TRN2", target_bir_lowering=False, debug=False,
                   num_devices=N_CORES)

    nodesT_d = nc.dram_tensor("nodesT", [IN_DIM, N_NODES], F16,
                              kind="ExternalInput")
    maskm_d = nc.dram_tensor("maskm", [N_NODES, ROWS], mybir.dt.float8e4,
                             kind="ExternalInput")
    wtaug_d = nc.dram_tensor("wt_aug", [IN_DIM, WTW], F16,
                             kind="ExternalInput")
    v1_d = nc.dram_tensor("v1t", [128, 4], F16, kind="ExternalInput")
    wc_d = nc.dram_tensor("wconst", [128, OUT_DIM + 3], F32,
                          kind="ExternalInput")
    out_d = nc.dram_tensor("out", [ROWS, OUT_DIM], F32, kind="ExternalOutput")

    with tile.TileContext(nc) as tc:
        with (
            tc.tile_pool(name="consts", bufs=1) as consts,
            tc.tile_pool(name="ndpool", bufs=3) as ndpool,
            tc.tile_pool(name="grpp", bufs=4) as grpp,
            tc.tile_pool(name="outp", bufs=2) as outp,
        ):
            # ---- persistent tiles ----
            wh_aug = consts.tile([128, NCK, WAUG], F16)
            wh2f32 = consts.tile([128, NCK], F32)
            q128 = consts.tile([128, NCK], F32)
            r128 = consts.tile([128, NCK], F32)
            p_row = consts.tile([1, ROWS], F16)
            p_b = consts.tile([128, ROWS], F16)
            warm_a = consts.tile([128, 64], F16)
            nc.gpsimd.memset(warm_a[:], 0.0)
            # denominator column = 1, rider column = 0, for all chunks
            nc.gpsimd.memset(wh_aug[:, :, OUT_DIM:OUT_DIM + 1], 1.0)
            nc.gpsimd.memset(wh_aug[:, :, OUT_DIM + 1:OUT_DIM + 2], 0.0)

            # ---- critical-path DMAs first.  pw1 depends only on sync-issued
            # data (v1 + both ndT0 halves); builds need wt (scalar, first).
            ndT_tiles = {}

            def dma_ndT(b, engines=(None, None)):
                e0 = engines[0] or nc.sync
                e1 = engines[1] or nc.scalar
                ndT = ndpool.tile([128, 4, ROWS], F16, name="ndT", tag="ndT")
                for half, eng in ((0, e0), (1, e1)):
                    src = nodesT_d[half * 256:(half + 1) * 256,
                                   b * ROWS:(b + 1) * ROWS]
                    src = src.rearrange("(d p) n -> p d n", p=128)
                    eng.dma_start(ndT[:, half * 2:half * 2 + 2, :], src)
                ndT_tiles[b] = ndT

            v1t = consts.tile([128, 4], F16)
            nc.sync.dma_start(v1t[:], v1_d[:])
            wt_t = []
            for d4 in range(4):
                w = consts.tile([128, WTW], F16, name=f"wt{d4}",
                                tag=f"wt{d4}")
                nc.scalar.dma_start(w[:], wtaug_d[d4 * 128:(d4 + 1) * 128, :])
                wt_t.append(w)
            dma_ndT(0, engines=(nc.sync, nc.sync))
            # wconst: cols 0:256 = W_b broadcast, 256:259 = cb consts
            wconst = consts.tile([128, OUT_DIM + 3], F32)
            nc.sync.dma_start(wconst[:], wc_d[:])
            wb_bc = wconst[:, 0:OUT_DIM]
            cb = wconst[:, OUT_DIM:OUT_DIM + 3]

            dma_ndT(1)

            # ---- mask DMA helper + prefetch of the first two groups ----
            mask_tiles = {}

            def dma_mask(h, g, suffix="", bufs=4):
                mgrp = grpp.tile([128, GRP, HALF], F16, name=f"mgrp{suffix}",
                                 tag=f"mgrp{suffix}", bufs=bufs)
                msrc = maskm_d[g * GRP * 128:(g + 1) * GRP * 128,
                               h * HALF:(h + 1) * HALF]
                msrc = msrc.rearrange("(c p) i -> p c i", p=128)
                nc.gpsimd.dma_start(mgrp[:], msrc)  # fp8->fp16 cast
                mask_tiles[(h, g)] = mgrp
                return mgrp

            dma_mask(0, 0)
            dma_mask(0, 1)

            with (
                tc.tile_pool(name="psW", bufs=1, space="PSUM") as psW,
                tc.tile_pool(name="psH", bufs=3, space="PSUM") as psH,
                tc.tile_pool(name="psB", bufs=1, space="PSUM") as psB,
            ):
                # ---- PE warm-up: dummy matmuls on zeroed scratch ----
                warm_ps = psW.tile([1, HALF], F32, name="warm_ps", tag="pw1")
                for k in range(WARM_MMS):
                    nc.tensor.matmul(
                        warm_ps[:, 0:64], warm_a[:, 0:1], warm_a[:, 0:64],
                        start=(k == 0), stop=(k == WARM_MMS - 1),
                        skip_group_check=True)

                ndT0 = ndT_tiles[0]

                # ---- Wh1 row for own block; p = exp(0.8*Wh1) ----
                for h2 in range(2):
                    pw1 = psW.tile([1, HALF], F32, name="pw1", tag="pw1")
                    for d4 in range(4):
                        nc.tensor.matmul(
                            pw1[:], v1t[:, d4:d4 + 1],
                            ndT0[:, d4, h2 * HALF:(h2 + 1) * HALF],
                            start=(d4 == 0), stop=(d4 == 3),
                            skip_group_check=True)
                    nc.scalar.activation(
                        p_row[:, h2 * HALF:(h2 + 1) * HALF], pw1[:], EXP,
                        scale=1.0 - ALPHA, bias=cb[0:1, 0:1])
                    nc.gpsimd.partition_broadcast(
                        p_b[:, h2 * HALF:(h2 + 1) * HALF],
                        p_row[:, h2 * HALF:(h2 + 1) * HALF])

                def build_block(b, early=False):
                    ndT = ndT_tiles.pop(b)
                    for ckl in range(8):
                        ck = b * 8 + ckl
                        pwh = psH.tile([128, WAUG], F32, name="pwh",
                                       tag="pwh", bufs=3)
                        for d4 in range(4):
                            nc.tensor.matmul(
                                pwh[:],
                                ndT[:, d4, ckl * 128:(ckl + 1) * 128],
                                wt_t[d4][:, 0:WAUG],
                                start=(d4 == 0), stop=(d4 == 3),
                                skip_group_check=True)
                        # col 257 of pwh = Wh2 (wt_aug col 257 = W.T@a2)
                        if early:
                            nc.vector.tensor_copy(wh2f32[:, ck:ck + 1],
                                                  pwh[:, 257:258])
                        else:
                            nc.scalar.copy(wh2f32[:, ck:ck + 1],
                                           pwh[:, 257:258])
                        # value columns, plain fp32->fp16 copy
                        nc.scalar.copy(wh_aug[:, ck, 0:OUT_DIM],
                                       pwh[:, 0:OUT_DIM])
                    sl = slice(b * 8, b * 8 + 8)
                    nc.scalar.activation(q128[:, sl], wh2f32[:, sl], EXP,
                                         scale=ALPHA, bias=cb[:, 1:2])
                    nc.scalar.activation(r128[:, sl], wh2f32[:, sl], EXP,
                                         scale=1.0, bias=cb[:, 2:3])

                build_block(0, early=True)
                dma_ndT(2)
                build_block(1, early=True)

                def emit_x(h, g, suffix="", bufs=4):
                    mgrp = mask_tiles.pop((h, g), None)
                    if mgrp is None:
                        mgrp = dma_mask(h, g, suffix, bufs)
                        mask_tiles.pop((h, g))
                    sgrp = grpp.tile([128, GRP, HALF], F16,
                                     name=f"sgrp{suffix}",
                                     tag=f"sgrp{suffix}", bufs=bufs)
                    for ckl in range(GRP):
                        ck = g * GRP + ckl
                        nc.vector.tensor_scalar(
                            sgrp[:, ckl, :],
                            p_b[:, h * HALF:(h + 1) * HALF],
                            r128[:, ck:ck + 1], q128[:, ck:ck + 1],
                            op0=MULT, op1=MAX)
                    xgrp = grpp.tile([128, GRP, HALF], F16,
                                     name=f"xgrp{suffix}",
                                     tag=f"xgrp{suffix}",
                                     bufs=max(2, bufs - 1))
                    nc.vector.tensor_tensor(xgrp[:], sgrp[:], mgrp[:],
                                            op=MULT)
                    return xgrp

                def readout_ib(h, accs, ib, o4):
                    recip = outp.tile([128, 1], F32, name="recip",
                                      tag="recip", bufs=4)
                    nc.vector.reciprocal(
                        recip[:], accs[ib][:, OUT_DIM:OUT_DIM + 1])
                    nc.vector.scalar_tensor_tensor(
                        o4[:, ib, :], accs[ib][:, 0:OUT_DIM], recip[:],
                        wb_bc, op0=MULT, op1=ADD)
                    if ib % 2 == 1:
                        r0 = h * HALF + (ib - 1) * 128
                        dst = out_d[r0:r0 + 256, :]
                        dst = dst.rearrange("(b p) k -> p b k", p=128)
                        eng = nc.sync if ib == 1 else nc.scalar
                        eng.dma_start(dst, o4[:, ib - 1:ib + 1, :])

                # build schedule: blocks 0-2 handled above; block b (3..7)
                # DMA'd at group b-3, built at group b-2.
                prefetched = []
                for h in range(2):
                    accs = [psB.tile([128, WAUG], F32, name=f"acc{ib}",
                                     tag=f"acc{ib}") for ib in range(4)]
                    for g in range(NG):
                        if (h, g + 2) not in mask_tiles and g + 2 < NG:
                            dma_mask(h, g + 2)
                        if h == 0 and g >= NG - 2:
                            # stage h1's prefetch-group masks early
                            dma_mask(1, g - (NG - 2), "p", 2)
                        if h == 0 and g < 6:
                            if g + 3 <= 7:
                                dma_ndT(g + 3)
                            build_block(g + 2)
                        if h == 1 and g < len(prefetched):
                            xgrp = prefetched[g]
                        else:
                            xgrp = emit_x(h, g)
                        last_g = (g == NG - 1)
                        if last_g:
                            # ib-outer so acc banks retire in order
                            for ib in range(4):
                                for ckl in range(GRP):
                                    ck = g * GRP + ckl
                                    nc.tensor.matmul(
                                        accs[ib][:],
                                        xgrp[:, ckl, ib * 128:(ib + 1) * 128],
                                        wh_aug[:, ck, :],
                                        start=False, stop=(ckl == GRP - 1),
                                        skip_group_check=True)
                        else:
                            for ckl in range(GRP):
                                ck = g * GRP + ckl
                                for ib in range(4):
                                    nc.tensor.matmul(
                                        accs[ib][:],
                                        xgrp[:, ckl, ib * 128:(ib + 1) * 128],
                                        wh_aug[:, ck, :],
                                        start=(ck == 0), stop=False,
                                        skip_group_check=True)
                    o4 = outp.tile([128, 4, OUT_DIM], F32, name="o4",
                                   tag="o4", bufs=2)
                    if h == 0:
                        # build h=1's first two score groups around the h0
                        # readout so neither blocks the other on the DVE
                        prefetched = [emit_x(1, 0, "p", 2)]
                        readout_ib(0, accs, 0, o4)
                        readout_ib(0, accs, 1, o4)
                        prefetched.append(emit_x(1, 1, "p", 2))
                        readout_ib(0, accs, 2, o4)
                        readout_ib(0, accs, 3, o4)
                    else:
                        for ib in range(4):
                            readout_ib(1, accs, ib, o4)
    nc.compile()
    return nc


def _get_nc():
    if "nc" not in _CACHE:
        _CACHE["nc"] = _build_nc()
    return _CACHE["nc"]


def _prep_in_maps(nodes, edge_mat, W_w, W_b, a1_w, a1_b, a2_w, a2_b):
    f16 = np.float16
    nodes = np.asarray(nodes, dtype=np.float32)
    edge_mat = np.asarray(edge_mat, dtype=bool)
    W_w = np.asarray(W_w, dtype=np.float32)
    W_b = np.asarray(W_b, dtype=np.float32)
    a1_w = np.asarray(a1_w, dtype=np.float32)
    a1_b = np.asarray(a1_b, dtype=np.float32)
    a2_w = np.asarray(a2_w, dtype=np.float32)
    a2_b = np.asarray(a2_b, dtype=np.float32)

    nodesT = np.ascontiguousarray(nodes.T).astype(f16)          # [512, 8192]
    v1 = (W_w.T @ a1_w[0]).astype(f16)[:, None]                 # [512, 1]
    v2 = (W_w.T @ a2_w[0]).astype(f16)[:, None]
    zc = np.zeros((IN_DIM, 1), f16)
    # cols: 0:256 = W.T, 256 = 0 (denom slot), 257 = v2, 258 = v1, 259 = 0
    wt_aug = np.concatenate([W_w.T.astype(f16), zc, v2, v1, zc], axis=1)
    c1v = float(W_b @ a1_w[0]) + float(a1_b[0])
    c2v = float(W_b @ a2_w[0]) + float(a2_b[0])
    # wconst cols 0:256 = W_b bcast; 256 = p bias; 257 = q bias; 258 = r bias
    wconst = np.concatenate([
        np.broadcast_to(W_b[None, :], (128, OUT_DIM)),
        np.broadcast_to(np.array(
            [(1.0 - ALPHA) * c1v, ALPHA * c2v, c2v],
            np.float32)[None, :], (128, 3)),
    ], axis=1).astype(np.float32)
    # multiplicative {0,1} mask, transposed, fp8 (cast to fp16 during DMA)
    import ml_dtypes
    maskT = np.where(edge_mat, 1, 0).astype(ml_dtypes.float8_e4m3fn).T

    in_maps = []
    for c in range(N_CORES):
        rs = c * ROWS
        sl = slice(rs, rs + ROWS)
        # roll node columns / mask rows so each core's own block is first
        nodesT_c = np.ascontiguousarray(
            np.concatenate([nodesT[:, rs:], nodesT[:, :rs]], axis=1))
        maskm_c = np.ascontiguousarray(
            np.concatenate([maskT[rs:, sl], maskT[:rs, sl]], axis=0))
        in_maps.append({
            "nodesT": nodesT_c,
            "maskm": maskm_c,
            "wt_aug": wt_aug,
            "v1t": np.ascontiguousarray(v1[:, 0].reshape(4, 128).T),
            "wconst": wconst,
        })
    return in_maps


def _run(inputs, trace=False, trace_cores=None):
    from concourse.bass_utils import run_bass_kernel_spmd
    if trace:
        _ensure_ntff_hook()
    nc = _get_nc()
    in_maps = _prep_in_maps(**inputs)
    res = run_bass_kernel_spmd(nc, in_maps, list(range(N_CORES)),
                               trace=trace, trace_cores=trace_cores)
    out = np.concatenate([res.results[c]["out"] for c in range(N_CORES)],
                         axis=0)
    return out, res


def kernel(**inputs) -> np.ndarray:
    out, _ = _run(inputs, trace=False)
    return out
